# revision 1
# baseline (speedup 1.0000x reference)
"""Trainium2 Bass kernel for nn_EnhancedHBitLinear.

Computation (per reference.py):
  x [2, 4096, 4096] -> flatten tokens T=8192
  xh = FWHT_4096(x) / 64 * had_scale + had_shift
  gamma[t] = max|xh[t,:]| + 1e-5 ; q = round(xh * 7/gamma)  (int4 levels)
  wscale = mean|W| + 1e-5 ; tern = clip(round(W/wscale), -1, 1)
  out[t,o] = sum_i q[t,i]*tern[o,i] * (gamma[t]/7) * wscale

Sharding: Megatron column-parallel. weight split into 8 shards of 2048 output
features; x / had_* replicated. Each core computes the full activation
pipeline + its out-column shard; host concatenates shards.

Device dataflow (per core), features-on-partitions layout throughout:
  - host passes x transposed: xt [4096, 8192] (layout prep only)
  - F1: per 128-feature chunk c, PE matmul with H128 (fp32) -> z[c] in PSUM
        (H4096 = H32(chunk bits) (x) H128(low bits))
  - shuffle: SBUF->SBUF DMA regroups partitions so chunk index is on
        partitions: z2[(lo4,c), (lohi,t)]
  - F2: PE matmul with (I4 (x) H32) -> full FWHT, feature order permuted;
        ACT evacuates PSUM fusing had_scale/64 (scale) + had_shift (bias)
  - gamma: gpsimd abs_max tree over lohi + partition_all_reduce(max)
  - quant: DVE mult by 7/gamma (free-dim stride-0 broadcast) + round via
        +/-1.5*2^23 magic (round-half-even, matches jnp.round), cast fp8e4.
        Integers in [-7,7] are exact in fp8e4.
  - weights: ternarized on device ({-1,0,1} via compares vs +-wscale/2),
        cast fp8e4; global wscale via AllReduce collective across 8 cores.
  - main matmul: fp8 x fp8 PE matmul, fp32 PSUM accumulation is EXACT
        (integer sums <= 7*4096); ACT evac applies gamma*wscale/7 per token.

kernel(**inputs) is self-contained: hardcodes shapes, shards internally,
runs on 8 NeuronCores via run_bass_kernel_spmd, gathers the full output.
"""

import math
import os
import sys

import numpy as np

sys.path.insert(0, "/opt/trn_rl_repo")

import concourse.bass as bass
import concourse.mybir as mybir
import concourse.tile as tile
from concourse import bass_isa, library_config
from concourse.bass_utils import run_bass_kernel_spmd

F32 = mybir.dt.float32
FP8 = mybir.dt.float8e4

IN = 4096
OUT = 16384
N_CORES = 8
OSH = OUT // N_CORES  # 2048 out features per core
T_FULL = 8192

CH = 32   # feature chunks of 128 (IN/128)
LOHI = 32
LO4 = 4
EPS = 1e-5
ACT_QB = 7.0
MAGIC = 12582912.0  # 1.5 * 2**23 : v+M-M == round-half-even(v) for |v| < 2**22


def _hadamard(n):
    h = np.array([[1.0]], dtype=np.float32)
    while h.shape[0] < n:
        h = np.block([[h, h], [h, -h]])
    return h


def host_consts():
    """H128p: F1 stationary with output partitions permuted so partition
    m = lo4*32+lohi holds FWHT-low-bits index lo = lohi*4+lo4 (makes the
    shuffle DMA source slices partition-contiguous).
    S2: F2 stationary. S2[k=lo4p*32+cp, m=lo4o*32+co] = (lo4p==lo4o)*H32[co,cp].
    perm: flat feature permutation of the pipeline output:
    j = kk*128 + p2 (k-chunk kk=lohi, partition p2=lo4*32+c) -> original i."""
    H128 = _hadamard(128)
    m = np.arange(128)
    lo_of_m = (m % 32) * 4 + m // 32
    H128p = H128[:, lo_of_m].astype(np.float32).copy()

    H32 = _hadamard(32)
    # z2 partition p2 = c*4 + lo4 (contiguous partitions per chunk c)
    # S2[k = cp*4+lo4p, m = c*4+lo4] = (lo4p==lo4) * H32[c, cp]
    S2 = np.zeros((128, 128), dtype=np.float32)
    for cp in range(32):
        for c in range(32):
            for lo4 in range(LO4):
                S2[cp * 4 + lo4, c * 4 + lo4] = H32[c, cp]
    perm = np.zeros(IN, dtype=np.int64)
    for kk in range(CH):
        for p2 in range(128):
            c = p2 // 4
            lo4 = p2 % 4
            perm[kk * 128 + p2] = c * 128 + kk * 4 + lo4
    return H128p, S2, perm


def build_program(n_cores=N_CORES, T=T_FULL, osh=OSH, t_blk=128, grp_blks=8,
                  debug=False, use_collective=True):
    """Build the single SPMD Bass program (identical on all cores)."""
    assert T % t_blk == 0
    nblk = T // t_blk
    assert nblk % grp_blks == 0
    ngrp = nblk // grp_blks
    obw = min(512, osh)
    n_ob = osh // obw
    sl_per_blk = (LOHI * t_blk) // 512  # f2 512-wide slices per block
    lohi_per_sl = LOHI // sl_per_blk

    nc = bass.Bass("TRN2", target_bir_lowering=False, debug=debug,
                   num_devices=n_cores)

    xt_d = nc.dram_tensor("xt", [IN, T], F32, kind="ExternalInput")
    wt_d = nc.dram_tensor("wt", [IN, osh], F32, kind="ExternalInput")
    hs_d = nc.dram_tensor("hs2", [128, LOHI], F32, kind="ExternalInput")
    hb_d = nc.dram_tensor("hb2", [128, LOHI], F32, kind="ExternalInput")
    h1_d = nc.dram_tensor("h128p", [128, 128], F32, kind="ExternalInput")
    s2_d = nc.dram_tensor("s2", [128, 128], F32, kind="ExternalInput")
    id_d = nc.dram_tensor("id128", [128, 128], F32, kind="ExternalInput")
    out_d = nc.dram_tensor("out", [T, osh], F32, kind="ExternalOutput")

    AL = mybir.AluOpType
    AF = mybir.ActivationFunctionType

    with tile.TileContext(nc) as tc:
        from contextlib import ExitStack
        with ExitStack() as ctx:
            singles = ctx.enter_context(tc.tile_pool(name="singles", bufs=1))
            dramp = ctx.enter_context(
                tc.tile_pool(name="dramp", bufs=1, space=bass.MemorySpace.DRAM))

            wq_d = dramp.tile([CH, 128, osh], FP8)
            gam_d = dramp.tile([1, T], F32)
            cc_in = dramp.tile([1, 1], F32)
            cc_out = dramp.tile([1, 1], F32)
            nc._dbg_names = {"wq": wq_d.tensor.name, "gam": gam_d.tensor.name,
                             "cc_out": cc_out.tensor.name}

            # constants
            h1 = singles.tile([128, 128], F32)
            nc.sync.dma_start(out=h1, in_=h1_d[:, :])
            s2 = singles.tile([128, 128], F32)
            nc.sync.dma_start(out=s2, in_=s2_d[:, :])
            hs2 = singles.tile([128, LOHI], F32)
            nc.sync.dma_start(out=hs2, in_=hs_d[:, :])
            hb2 = singles.tile([128, LOHI], F32)
            nc.sync.dma_start(out=hb2, in_=hb_d[:, :])
            id128 = singles.tile([128, 128], F32)
            nc.sync.dma_start(out=id128, in_=id_d[:, :])
            ones128 = singles.tile([128, 1], F32)
            nc.vector.memset(ones128, 1.0)
            hs2p = singles.tile([128, LOHI], F32)
            nc.vector.tensor_scalar_mul(out=hs2p, in0=hs2, scalar1=1.0 / 64.0)
            sev = singles.tile([128, t_blk], F32)
            nc.vector.memset(sev, ACT_QB)

            # ---------------- weight prep ----------------
            scl128 = singles.tile([128, 1], F32)   # wscale bcast
            thr128 = singles.tile([128, 1], F32)   # +wscale/2
            nthr128 = singles.tile([128, 1], F32)  # -wscale/2
            ws7 = singles.tile([128, 1], F32)      # wscale/7
            with tc.tile_pool(name="wprep", bufs=2) as wp, \
                 tc.tile_pool(name="wacc", bufs=1) as wa:
                accs = wa.tile([128, CH], F32)
                for kk in range(CH):
                    wf = wp.tile([128, osh], F32, tag="wf")
                    nc.sync.dma_start(out=wf,
                                      in_=wt_d[kk * 128:(kk + 1) * 128, :])
                    nc.vector.tensor_reduce(
                        out=accs[:, kk:kk + 1], in_=wf,
                        axis=mybir.AxisListType.X, op=AL.add,
                        apply_absolute_value=True)
                tot128 = wa.tile([128, 1], F32)
                nc.vector.tensor_reduce(out=tot128, in_=accs,
                                        axis=mybir.AxisListType.X, op=AL.add)
                with tc.tile_pool(name="wps", bufs=1, space="PSUM") as wps:
                    tot1p = wps.tile([1, 1], F32)
                    nc.tensor.matmul(tot1p, lhsT=ones128, rhs=tot128,
                                     start=True, stop=True)
                    tot1 = wa.tile([1, 1], F32)
                    nc.scalar.copy(tot1, tot1p)
                nc.sync.dma_start(out=cc_in[:, :], in_=tot1)
                if use_collective:
                    nc.gpsimd.collective_compute(
                        "AllReduce", AL.add,
                        replica_groups=[list(range(n_cores))],
                        ins=[cc_in[:, :]], outs=[cc_out[:, :]])
                else:
                    nc.sync.dma_start(out=cc_out[:, :], in_=cc_in[:, :])
                totg = wa.tile([128, 1], F32)
                cc_bcast = bass.AP(tensor=cc_out.tensor, offset=cc_out.offset,
                                   ap=[[0, 128], [1, 1]])
                nc.gpsimd.dma_start(out=totg, in_=cc_bcast)
                # wscale = mean + EPS ; mean over the FULL weight
                nc.vector.tensor_scalar(
                    out=scl128, in0=totg, scalar1=1.0 / (IN * osh * n_cores),
                    scalar2=EPS, op0=AL.mult, op1=AL.add)
                nc.vector.tensor_scalar_mul(out=thr128, in0=scl128, scalar1=0.5)
                nc.vector.tensor_scalar_mul(out=nthr128, in0=scl128,
                                            scalar1=-0.5)
                nc.vector.tensor_scalar_mul(out=ws7, in0=scl128,
                                            scalar1=1.0 / 7.0)
                for kk in range(CH):
                    wf = wp.tile([128, osh], F32, tag="wf")
                    nc.sync.dma_start(out=wf,
                                      in_=wt_d[kk * 128:(kk + 1) * 128, :])
                    g = wp.tile([128, osh], F32, tag="g")
                    nc.vector.tensor_scalar(out=g, in0=wf, scalar1=thr128,
                                            scalar2=None, op0=AL.is_gt)
                    l = wp.tile([128, osh], F32, tag="l")
                    nc.vector.tensor_scalar(out=l, in0=wf, scalar1=nthr128,
                                            scalar2=None, op0=AL.is_lt)
                    tq = wp.tile([128, osh], FP8, tag="tq")
                    nc.vector.tensor_tensor(out=tq, in0=g, in1=l,
                                            op=AL.subtract)
                    nc.sync.dma_start(out=wq_d[kk, :, :], in_=tq)

            # ---------------- main pipeline ----------------
            xp = ctx.enter_context(tc.tile_pool(name="xp", bufs=2))
            zp = ctx.enter_context(tc.tile_pool(name="zp", bufs=1))
            z2p = ctx.enter_context(tc.tile_pool(name="z2p", bufs=2))
            y2p = ctx.enter_context(tc.tile_pool(name="y2p", bufs=1))
            qp = ctx.enter_context(tc.tile_pool(name="qp", bufs=5))
            wgp = ctx.enter_context(tc.tile_pool(name="wgp", bufs=CH + 1))
            op_ = ctx.enter_context(tc.tile_pool(name="op", bufs=2))
            gp = ctx.enter_context(tc.tile_pool(name="gp", bufs=4))
            sp = ctx.enter_context(tc.tile_pool(name="sp", bufs=4))
            psf1 = ctx.enter_context(
                tc.tile_pool(name="psf1", bufs=2, space="PSUM"))
            psf2 = ctx.enter_context(
                tc.tile_pool(name="psf2", bufs=2, space="PSUM"))
            psm = ctx.enter_context(
                tc.tile_pool(name="psm", bufs=1, space="PSUM"))

            for grp in range(ngrp):
                q_tiles = []
                for b in range(grp_blks):
                    blk = grp * grp_blks + b
                    t0 = blk * t_blk
                    z2t = z2p.tile([128, LOHI * t_blk], F32, tag="z2")
                    z2v = z2t.rearrange("(c f) (h t) -> c f h t",
                                        f=LO4, h=LOHI)
                    zbig = zp.tile([128, CH * t_blk], F32, tag="zbig")
                    zbv = zbig.rearrange("p (c t) -> p c t", c=CH)
                    n_xg = CH // 8
                    for xg in range(n_xg):
                        x_t = xp.tile([128, 8 * t_blk], F32, tag="x")
                        # one DMA for 8 feature-chunks: dst (p, c8, t)
                        nc.sync.dma_start(
                            out=x_t.rearrange("p (c t) -> p c t", c=8),
                            in_=xt_d[:, t0:t0 + t_blk].rearrange(
                                "(cc p) t -> p cc t", p=128)[
                                    :, xg * 8:(xg + 1) * 8, :])
                        for c8 in range(8):
                            c = xg * 8 + c8
                            pf1 = psf1.tile([128, t_blk], F32, tag="pf1")
                            nc.tensor.matmul(
                                pf1, lhsT=h1,
                                rhs=x_t[:, c8 * t_blk:(c8 + 1) * t_blk],
                                start=True, stop=True)
                            nc.scalar.copy(zbv[:, c, :], pf1)
                            # shuffle: z2[(c,lo4), lohi, t] = z[lo4*32+lohi, t]
                            nc.sync.dma_start(out=z2v[c], in_=zbv[:, c, :])
                    y2t = y2p.tile([128, LOHI * t_blk], F32, tag="y2")
                    for s in range(sl_per_blk):
                        pf2 = psf2.tile([128, 512], F32, tag="pf2")
                        nc.tensor.matmul(pf2, lhsT=s2,
                                         rhs=z2t[:, s * 512:(s + 1) * 512],
                                         start=True, stop=True)
                        for j in range(lohi_per_sl):
                            lohi = s * lohi_per_sl + j
                            nc.scalar.activation(
                                out=y2t[:, lohi * t_blk:(lohi + 1) * t_blk],
                                in_=pf2[:, j * t_blk:(j + 1) * t_blk],
                                func=AF.Identity,
                                scale=hs2p[:, lohi:lohi + 1],
                                bias=hb2[:, lohi:lohi + 1])
                    # gamma: abs-max over lohi (strided reduce), then over
                    # partitions via PE transpose + free-dim reduce
                    rmax = gp.tile([128, t_blk], F32, tag="rmax")
                    nc.vector.tensor_reduce(
                        out=rmax,
                        in_=y2t.rearrange("p (h t) -> p t h", h=LOHI),
                        axis=mybir.AxisListType.X, op=AL.max,
                        apply_absolute_value=True)
                    rmaxT = psf1.tile([128, t_blk], F32, tag="pf1")
                    nc.tensor.transpose(rmaxT[0:t_blk, :], rmax, id128)
                    gam = gp.tile([t_blk, 1], F32, tag="gam")
                    nc.vector.tensor_reduce(
                        out=gam, in_=rmaxT[0:t_blk, :],
                        axis=mybir.AxisListType.X, op=AL.max)
                    nc.vector.tensor_scalar_add(out=gam, in0=gam, scalar1=EPS)
                    # gamma[t] (t on partitions, eps included) -> DRAM
                    nc.sync.dma_start(
                        out=gam_d[0:1, t0:t0 + t_blk].rearrange(
                            "o (p q) -> (o p) q", q=1),
                        in_=gam)
                    # broadcast gamma along partitions for the quant multiply
                    gamb = gp.tile([128, t_blk], F32, tag="gamb")
                    gam_bc = bass.AP(
                        tensor=gam_d.tensor, offset=gam_d.offset + t0,
                        ap=[[0, 128], [1, t_blk]])
                    nc.gpsimd.dma_start(out=gamb, in_=gam_bc)
                    sb = gp.tile([128, t_blk], F32, tag="sb")
                    nc.vector.reciprocal(out=sb, in_=gamb)
                    nc.vector.tensor_scalar_mul(out=sb, in0=sb,
                                                scalar1=ACT_QB)
                    # quant: y2 *= sb (broadcast over lohi), round, cast fp8
                    y2v = y2t.rearrange("p (h t) -> p h t", h=LOHI)
                    sbv = sb.rearrange("p (o t) -> p o t", o=1)
                    a1, a2 = bass.broadcast_tensor_aps(y2v, sbv)
                    nc.vector.tensor_tensor(out=y2v, in0=a1, in1=a2,
                                            op=AL.mult)
                    qb = qp.tile([128, LOHI * t_blk], FP8, tag="q")
                    nc.vector.tensor_scalar(out=qb, in0=y2t, scalar1=MAGIC,
                                            scalar2=MAGIC, op0=AL.add,
                                            op1=AL.subtract)
                    q_tiles.append(qb)
                # ---- main matmul over this group ----
                wg_tiles = []
                for kk in range(CH):
                    wg = wgp.tile([128, osh], FP8, tag="wg")
                    nc.sync.dma_start(out=wg, in_=wq_d[kk, :, :])
                    wg_tiles.append(wg)
                for b in range(grp_blks):
                    blk = grp * grp_blks + b
                    t0 = blk * t_blk
                    gt = sp.tile([128, 1], F32, tag="gt")
                    nc.sync.dma_start(
                        out=gt,
                        in_=gam_d[0:1, t0:t0 + t_blk].rearrange(
                            "o (p q) -> (o p) q", q=1))
                    so = sp.tile([128, 1], F32, tag="so")
                    nc.vector.tensor_tensor(out=so, in0=gt, in1=ws7,
                                            op=AL.mult)
                    psums = [psm.tile([128, obw], F32, tag=f"pm{ob}",
                                      name=f"pm{ob}")
                             for ob in range(n_ob)]
                    qb = q_tiles[b]
                    for kk in range(CH):
                        lhs = qb[:, kk * t_blk:(kk + 1) * t_blk]
                        for ob in range(n_ob):
                            nc.tensor.matmul(
                                psums[ob], lhsT=lhs,
                                rhs=wg_tiles[kk][:, ob * obw:(ob + 1) * obw],
                                start=(kk == 0), stop=(kk == CH - 1))
                    ot = op_.tile([128, osh], F32, tag="ot")
                    for ob in range(n_ob):
                        nc.scalar.activation(
                            out=ot[:, ob * obw:(ob + 1) * obw],
                            in_=psums[ob], func=AF.Copy, scale=so)
                    nc.sync.dma_start(out=out_d[t0:t0 + t_blk, :], in_=ot)

    return nc


def _split_multi_waits(nc):
    """walrus's CTRL encoder fits one sem-wait per instruction; Tile can emit
    several (e.g. the kernel-tail drain). Hoist extras onto standalone
    InstEventSemaphore carriers inserted just before the instruction."""
    import copy

    m = nc.m
    new_module = copy.replace(m, functions=[])
    ctr = 0
    for function in m.functions:
        new_function = copy.replace(function, blocks=[])
        new_function.set_allocations_from_list(function.allocations)
        for block in function.blocks:
            new_insts = []
            for inst in block.instructions:
                si = inst.sync_info
                ow = list(si.on_wait) if si is not None and si.on_wait else []
                if len(ow) > 1:
                    for w in ow[:-1]:
                        ctr += 1
                        new_insts.append(mybir.InstEventSemaphore(
                            name=f"I-wsplit-{ctr}",
                            engine=inst.engine,
                            ins=[], outs=[],
                            sync_info=mybir.SyncInfo(on_wait=[w],
                                                     on_update=[])))
                    inst = copy.replace(
                        inst,
                        sync_info=mybir.SyncInfo(on_wait=[ow[-1]],
                                                 on_update=si.on_update))
                new_insts.append(inst)
            new_block = copy.replace(block, instructions=new_insts)
            new_function.blocks.append(new_block)
        new_module.functions.append(new_function)
    nc.m = new_module
    return ctr


def host_prep(x, weight, had_scale, had_shift, n_cores=N_CORES, osh=None):
    """Shard + re-layout inputs for the SPMD program. Layout prep only."""
    T = int(np.prod(x.shape[:-1]))
    osh = osh or weight.shape[0] // n_cores
    H128p, S2, perm = host_consts()
    xt = np.ascontiguousarray(x.reshape(T, IN).T)  # [4096, T]
    id128 = np.eye(128, dtype=np.float32)
    hs2 = np.ascontiguousarray(
        had_scale[perm].reshape(CH, 128).T)  # [128(p2), 32(kk)]
    hb2 = np.ascontiguousarray(had_shift[perm].reshape(CH, 128).T)
    in_maps = []
    for core in range(n_cores):
        wsh = weight[core * osh:(core + 1) * osh, :]  # [osh, IN]
        wt = np.ascontiguousarray(wsh[:, perm].T)     # [IN(perm j), osh]
        in_maps.append({
            "xt": xt, "wt": wt, "hs2": hs2, "hb2": hb2,
            "h128p": H128p, "s2": S2, "id128": id128,
        })
    return in_maps


_PROGRAM_CACHE = {}


def _get_program(key, **kwargs):
    if key not in _PROGRAM_CACHE:
        nc = build_program(**kwargs)
        _split_multi_waits(nc)
        _PROGRAM_CACHE[key] = nc
    return _PROGRAM_CACHE[key]


def _timed_pjrt(nc, in_maps, n_cores, iters=5):
    """Build a cached shard_map executable and time steady-state runs.

    Mirrors bass2jax.run_bass_via_pjrt but without output donation so the
    device-resident inputs/zero-outputs can be reused across timing runs.
    """
    import time

    import jax
    from jax.sharding import Mesh, NamedSharding, PartitionSpec
    from jax.experimental.shard_map import shard_map

    from concourse import bass2jax, mybir as mb

    bass2jax.install_neuronx_cc_hook()

    partition_name = (nc.partition_id_tensor.name
                      if nc.partition_id_tensor else None)
    in_names, out_names, out_avals, zero_outs = [], [], [], []
    for alloc in nc.m.functions[0].allocations:
        if not isinstance(alloc, mb.MemoryLocationSet):
            continue
        name = alloc.memorylocations[0].name
        if alloc.kind == "ExternalInput":
            if name != partition_name:
                in_names.append(name)
        elif alloc.kind == "ExternalOutput":
            out_names.append(name)
            shape = tuple(alloc.tensor_shape)
            dtype = mb.dt.np(alloc.dtype)
            out_avals.append(jax.core.ShapedArray(shape, dtype))
            zero_outs.append(np.zeros(shape, dtype))
    n_params = len(in_names)
    all_in_names = list(in_names) + list(out_names)
    if partition_name is not None:
        all_in_names.append(partition_name)

    def _body(*args):
        operands = list(args)
        if partition_name is not None:
            operands.append(bass2jax.partition_id_tensor())
        outs = bass2jax._bass_exec_p.bind(
            *operands,
            out_avals=tuple(out_avals),
            in_names=tuple(all_in_names),
            out_names=tuple(out_names),
            lowering_input_output_aliases=(),
            sim_require_finite=True,
            sim_require_nnan=True,
            nc=nc,
        )
        return tuple(outs)

    import jax.numpy as jnp

    devices = jax.devices()[:n_cores]
    mesh = Mesh(np.asarray(devices), ("core",))
    spec = NamedSharding(mesh, PartitionSpec("core"))
    n_outs = len(out_names)
    donate = tuple(range(n_params, n_params + n_outs))
    sharded = jax.jit(
        shard_map(_body, mesh=mesh,
                  in_specs=(PartitionSpec("core"),) * (n_params + n_outs),
                  out_specs=(PartitionSpec("core"),) * n_outs,
                  check_rep=False),
        donate_argnums=donate, keep_unused=True)

    concat_in = [
        np.concatenate([np.asarray(in_maps[c][nm]) for c in range(n_cores)],
                       axis=0)
        for nm in in_names
    ]
    dev_in = [jax.device_put(a, spec) for a in concat_in]
    zero_shapes = [(n_cores * z.shape[0], *z.shape[1:]) for z in zero_outs]
    make_zeros = jax.jit(
        lambda: tuple(jnp.zeros(s, z.dtype)
                      for s, z in zip(zero_shapes, zero_outs)),
        out_shardings=(spec,) * n_outs)

    dev_zero = make_zeros()
    jax.block_until_ready(dev_zero)
    out = sharded(*dev_in, *dev_zero)  # warmup/compile
    jax.block_until_ready(out)
    times = []
    for _ in range(iters):
        dev_zero = make_zeros()
        jax.block_until_ready(dev_zero)
        t0 = time.perf_counter()
        out = sharded(*dev_in, *dev_zero)
        jax.block_until_ready(out)
        times.append(time.perf_counter() - t0)
    results = [
        {nm: np.asarray(out[i]).reshape(n_cores, *out_avals[i].shape)[c]
         for i, nm in enumerate(out_names)}
        for c in range(n_cores)
    ]
    return results, times


def run(x, weight, had_scale, had_shift, trace=False, **trace_kwargs):
    x = np.asarray(x, dtype=np.float32)
    weight = np.asarray(weight, dtype=np.float32)
    had_scale = np.asarray(had_scale, dtype=np.float32)
    had_shift = np.asarray(had_shift, dtype=np.float32)
    batch_shape = x.shape[:-1]
    T = int(np.prod(batch_shape))
    nc = _get_program(("full", T), T=T)
    in_maps = host_prep(x, weight, had_scale, had_shift)
    if trace:
        results, times = _timed_pjrt(nc, in_maps, N_CORES)
    else:
        res = run_bass_kernel_spmd(nc, in_maps, core_ids=list(range(N_CORES)))
        results, times = res.results, None
    shards = [results[c]["out"] for c in range(N_CORES)]
    out = np.concatenate(shards, axis=1).reshape(*batch_shape, OUT)
    return out, times


def kernel(**inputs):
    out, _ = run(inputs["x"], inputs["weight"], inputs["had_scale"],
                 inputs["had_shift"])
    return out


if __name__ == "__main__":
    # smoke: build only
    nc = build_program(T=256, grp_blks=2)
    print("build ok:", len(nc.m.functions[0].basic_blocks[0].instructions)
          if hasattr(nc.m.functions[0], "basic_blocks") else "n/a")



# revision 7
# speedup vs baseline: 14.5790x; 14.5790x over previous
"""Trainium2 Bass kernel for nn_EnhancedHBitLinear.

Computation (per reference.py):
  x [2, 4096, 4096] -> flatten tokens T=8192
  xh = FWHT_4096(x) / 64 * had_scale + had_shift
  gamma[t] = max|xh[t,:]| + 1e-5 ; q = round(xh * 7/gamma)  (int4 levels)
  wscale = mean|W| + 1e-5 ; tern = clip(round(W/wscale), -1, 1)
  out[t,o] = sum_i q[t,i]*tern[o,i] * (gamma[t]/7) * wscale

Sharding: Megatron column-parallel. weight split into 8 shards of 2048 output
features; x / had_* replicated. Each core computes the full activation
pipeline + its out-column shard; host concatenates shards.

Device dataflow (per core), features-on-partitions layout throughout:
  - host passes x transposed: xt [4096, 8192] (layout prep only)
  - weights: ternarized on device ({-1,0,1} via compares vs +-wscale/2),
        cast fp8e4, kept RESIDENT in SBUF as 16 chunk-pair tiles
        [128, 2*osh]; global wscale via AllReduce across 8 cores.
  - per 128-token block:
    - F1: 8 PE matmuls (H128 stationary, N=512 = 4 feature chunks) -> ACT
          evacuates into zbig [128, (c,t)]
    - shuffle: 32 SBUF->SBUF DMAs regroup partitions so chunk index is on
          partitions: z2[(c,lo4), (lohi,t)]
    - F2: 8 PE matmuls with (I4 (x) H32) -> full FWHT (feature-permuted);
          ACT evacuates PSUM fusing had_scale/64 (scale) + had_shift (bias)
    - gamma: DVE strided abs-max over lohi -> rmax [128,t]; PE transpose;
          DVE max over partitions -> gam [t,1] (+eps). All on-chip.
    - sb broadcast: PE transpose gam -> [1,t]; reciprocal*7; K=1 matmul
          with ones -> sb [128,t] (7/gamma broadcast over partitions)
    - quant: DVE mult y2*sb (stride-0 bcast over lohi) + round via
          +/-1.5*2^23 magic (round-half-even, matches jnp.round), cast fp8e4.
    - main matmul: fp8 DoubleRow (chunk pairs, contraction 256/pass) into
          4 PSUM banks; fp32 accumulation is EXACT (integer sums <= 7*4096);
          ACT evac applies gamma*wscale/7 per token (per-partition scale).

kernel(**inputs) is self-contained: hardcodes shapes, shards internally,
runs on 8 NeuronCores via run_bass_kernel_spmd, gathers the full output.
"""

import math
import os
import sys

import numpy as np

sys.path.insert(0, "/opt/trn_rl_repo")

import concourse.bass as bass
import concourse.mybir as mybir
import concourse.tile as tile
from concourse import bass_isa, library_config
from concourse.bass_utils import run_bass_kernel_spmd

F32 = mybir.dt.float32
FP8 = mybir.dt.float8e4

IN = 4096
OUT = 16384
N_CORES = 8
OSH = OUT // N_CORES  # 2048 out features per core
T_FULL = 8192

CH = 32   # feature chunks of 128 (IN/128)
LOHI = 32
LO4 = 4
EPS = 1e-5
ACT_QB = 7.0
MAGIC = 12582912.0  # 1.5 * 2**23 : v+M-M == round-half-even(v) for |v| < 2**22


def _hadamard(n):
    h = np.array([[1.0]], dtype=np.float32)
    while h.shape[0] < n:
        h = np.block([[h, h], [h, -h]])
    return h


def host_consts():
    """H128p: F1 stationary with output partitions permuted so partition
    m = lo4*32+lohi holds FWHT-low-bits index lo = lohi*4+lo4 (makes the
    shuffle DMA source slices partition-contiguous).
    S2: F2 stationary. S2[k=lo4p*32+cp, m=lo4o*32+co] = (lo4p==lo4o)*H32[co,cp].
    perm: flat feature permutation of the pipeline output:
    j = kk*128 + p2 (k-chunk kk=lohi, partition p2=lo4*32+c) -> original i."""
    H128 = _hadamard(128)
    m = np.arange(128)
    lo_of_m = (m % 32) * 4 + m // 32
    H128p = H128[:, lo_of_m].astype(np.float32).copy()

    H32 = _hadamard(32)
    # z2 partition p2 = c*4 + lo4 (contiguous partitions per chunk c)
    # S2[k = cp*4+lo4p, m = c*4+lo4] = (lo4p==lo4) * H32[c, cp]
    S2 = np.zeros((128, 128), dtype=np.float32)
    for cp in range(32):
        for c in range(32):
            for lo4 in range(LO4):
                S2[cp * 4 + lo4, c * 4 + lo4] = H32[c, cp]
    perm = np.zeros(IN, dtype=np.int64)
    for kk in range(CH):
        for p2 in range(128):
            c = p2 // 4
            lo4 = p2 % 4
            perm[kk * 128 + p2] = c * 128 + kk * 4 + lo4
    return H128p, S2, perm


def build_program(n_cores=N_CORES, T=T_FULL, osh=OSH, t_blk=128,
                  debug=False, use_collective=True, use_dr=True,
                  debug_taps=False):
    """Build the single SPMD Bass program (identical on all cores)."""
    assert T % t_blk == 0
    nblk = T // t_blk
    obw = 512
    n_ob = osh // obw
    sl_per_blk = (LOHI * t_blk) // 512  # 512-wide slices per block (F1/F2)
    lohi_per_sl = LOHI // sl_per_blk
    n_pair = CH // 2

    nc = bass.Bass("TRN2", target_bir_lowering=False, debug=debug,
                   num_devices=n_cores)

    taps = {}
    if debug_taps:
        taps["zbig"] = nc.dram_tensor("tap_zbig", [128, CH * t_blk], F32,
                                      kind="ExternalOutput")
        taps["y2"] = nc.dram_tensor("tap_y2", [128, LOHI * t_blk], F32,
                                    kind="ExternalOutput")
        taps["gam"] = nc.dram_tensor("tap_gam", [t_blk, 1], F32,
                                     kind="ExternalOutput")
        taps["sb"] = nc.dram_tensor("tap_sb", [128, t_blk], F32,
                                    kind="ExternalOutput")
        taps["qb"] = nc.dram_tensor("tap_qb", [128, CH * t_blk], F32,
                                    kind="ExternalOutput")
        taps["wg0"] = nc.dram_tensor("tap_wg0", [128, 2 * osh], F32,
                                     kind="ExternalOutput")

    xt_d = nc.dram_tensor("xt", [IN, T], F32, kind="ExternalInput")
    wt_d = nc.dram_tensor("wt", [IN, osh], F32, kind="ExternalInput")
    hs_d = nc.dram_tensor("hs2", [128, LOHI], F32, kind="ExternalInput")
    hb_d = nc.dram_tensor("hb2", [128, LOHI], F32, kind="ExternalInput")
    h1_d = nc.dram_tensor("h128p", [128, 128], F32, kind="ExternalInput")
    s2_d = nc.dram_tensor("s2", [128, 128], F32, kind="ExternalInput")
    id_d = nc.dram_tensor("id128", [128, 128], F32, kind="ExternalInput")
    out_d = nc.dram_tensor("out", [T, osh], F32, kind="ExternalOutput")

    AL = mybir.AluOpType
    AF = mybir.ActivationFunctionType

    with tile.TileContext(nc) as tc:
        from contextlib import ExitStack
        with ExitStack() as ctx:
            singles = ctx.enter_context(tc.tile_pool(name="singles", bufs=1))
            dramp = ctx.enter_context(
                tc.tile_pool(name="dramp", bufs=1, space=bass.MemorySpace.DRAM))

            cc_in = dramp.tile([1, 1], F32)
            cc_out = dramp.tile([1, 1], F32)

            # constants
            h1 = singles.tile([128, 128], F32)
            nc.sync.dma_start(out=h1, in_=h1_d[:, :])
            s2 = singles.tile([128, 128], F32)
            nc.sync.dma_start(out=s2, in_=s2_d[:, :])
            hs2 = singles.tile([128, LOHI], F32)
            nc.sync.dma_start(out=hs2, in_=hs_d[:, :])
            hb2 = singles.tile([128, LOHI], F32)
            nc.sync.dma_start(out=hb2, in_=hb_d[:, :])
            id128 = singles.tile([128, 128], F32)
            nc.sync.dma_start(out=id128, in_=id_d[:, :])
            ones1 = singles.tile([1, 128], F32)
            nc.vector.memset(ones1, 1.0)
            ones128 = singles.tile([128, 1], F32)
            nc.vector.memset(ones128, 1.0)
            hs2p = singles.tile([128, LOHI], F32)
            nc.vector.tensor_scalar_mul(out=hs2p, in0=hs2, scalar1=1.0 / 64.0)

            # resident ternarized weights: 16 chunk-pair tiles [128, 2*osh]
            wgp = ctx.enter_context(tc.tile_pool(name="wgp", bufs=1))
            wg_tiles = [wgp.tile([128, 2 * osh], FP8, tag=f"wg{i}",
                                 name=f"wg{i}")
                        for i in range(n_pair)]

            # ---------------- weight prep ----------------
            scl128 = singles.tile([128, 1], F32)   # wscale bcast
            thr128 = singles.tile([128, 1], F32)   # +wscale/2
            nthr128 = singles.tile([128, 1], F32)  # -wscale/2
            ws7 = singles.tile([128, 1], F32)      # wscale/7
            with tc.tile_pool(name="wprep", bufs=2) as wp, \
                 tc.tile_pool(name="wacc", bufs=1) as wa:
                accs = wa.tile([128, CH], F32)
                for kk in range(CH):
                    wf = wp.tile([128, osh], F32, tag="wf")
                    nc.scalar.dma_start(out=wf,
                                        in_=wt_d[kk * 128:(kk + 1) * 128, :])
                    nc.vector.tensor_reduce(
                        out=accs[:, kk:kk + 1], in_=wf,
                        axis=mybir.AxisListType.X, op=AL.add,
                        apply_absolute_value=True)
                tot128 = wa.tile([128, 1], F32)
                nc.vector.tensor_reduce(out=tot128, in_=accs,
                                        axis=mybir.AxisListType.X, op=AL.add)
                with tc.tile_pool(name="wps", bufs=1, space="PSUM") as wps:
                    tot1p = wps.tile([1, 1], F32)
                    nc.tensor.matmul(tot1p, lhsT=ones128, rhs=tot128,
                                     start=True, stop=True)
                    tot1 = wa.tile([1, 1], F32)
                    nc.scalar.copy(tot1, tot1p)
                nc.sync.dma_start(out=cc_in[:, :], in_=tot1)
                if use_collective:
                    nc.gpsimd.collective_compute(
                        "AllReduce", AL.add,
                        replica_groups=[list(range(n_cores))],
                        ins=[cc_in[:, :]], outs=[cc_out[:, :]])
                else:
                    nc.sync.dma_start(out=cc_out[:, :], in_=cc_in[:, :])
                totg = wa.tile([128, 1], F32)
                cc_bcast = bass.AP(tensor=cc_out.tensor, offset=cc_out.offset,
                                   ap=[[0, 128], [1, 1]])
                nc.gpsimd.dma_start(out=totg, in_=cc_bcast)
                # wscale = mean + EPS ; mean over the FULL weight
                nc.vector.tensor_scalar(
                    out=scl128, in0=totg, scalar1=1.0 / (IN * osh * n_cores),
                    scalar2=EPS, op0=AL.mult, op1=AL.add)
                nc.vector.tensor_scalar_mul(out=thr128, in0=scl128, scalar1=0.5)
                nc.vector.tensor_scalar_mul(out=nthr128, in0=scl128,
                                            scalar1=-0.5)
                nc.vector.tensor_scalar_mul(out=ws7, in0=scl128,
                                            scalar1=1.0 / 7.0)
                for kk in range(CH):
                    wf = wp.tile([128, osh], F32, tag="wf")
                    nc.scalar.dma_start(out=wf,
                                        in_=wt_d[kk * 128:(kk + 1) * 128, :])
                    g = wp.tile([128, osh], F32, tag="g")
                    nc.vector.tensor_scalar(out=g, in0=wf, scalar1=thr128,
                                            scalar2=None, op0=AL.is_gt)
                    l = wp.tile([128, osh], F32, tag="l")
                    nc.vector.tensor_scalar(out=l, in0=wf, scalar1=nthr128,
                                            scalar2=None, op0=AL.is_lt)
                    dst = wg_tiles[kk // 2][:, (kk % 2) * osh:(kk % 2 + 1) * osh]
                    nc.vector.tensor_tensor(out=dst, in0=g, in1=l,
                                            op=AL.subtract)

            # ---------------- main pipeline ----------------
            dbufs = 1 if debug_taps else 2
            xp = ctx.enter_context(tc.tile_pool(name="xp", bufs=dbufs))
            zp = ctx.enter_context(tc.tile_pool(name="zp", bufs=1))
            z2p = ctx.enter_context(tc.tile_pool(name="z2p", bufs=dbufs))
            y2p = ctx.enter_context(tc.tile_pool(name="y2p", bufs=dbufs))
            qp = ctx.enter_context(tc.tile_pool(name="qp", bufs=2))
            op_ = ctx.enter_context(tc.tile_pool(name="op", bufs=2))
            gp = ctx.enter_context(tc.tile_pool(name="gp", bufs=2))
            psf1 = ctx.enter_context(
                tc.tile_pool(name="psf1", bufs=2, space="PSUM"))
            psf2 = ctx.enter_context(
                tc.tile_pool(name="psf2", bufs=2, space="PSUM"))
            psm = ctx.enter_context(
                tc.tile_pool(name="psm", bufs=1, space="PSUM"))

            for blk in range(nblk):
                t0 = blk * t_blk
                # ---- x load: one DMA for all 32 chunks of this block
                x_t = xp.tile([128, CH * t_blk], F32, tag="x")
                nc.sync.dma_start(
                    out=x_t.rearrange("p (c t) -> p c t", c=CH),
                    in_=xt_d[:, t0:t0 + t_blk].rearrange(
                        "(cc p) t -> p cc t", p=128))
                # ---- F1: 8 matmuls of N=512 (4 chunks each)
                zbig = zp.tile([128, CH * t_blk], F32, tag="zbig")
                zbv = zbig.rearrange("p (c t) -> p c t", c=CH)
                z2t = z2p.tile([128, LOHI * t_blk], F32, tag="z2")
                z2v = z2t.rearrange("(c f) (h t) -> c f h t",
                                    f=LO4, h=LOHI)
                for g in range(sl_per_blk):
                    pf1 = psf1.tile([128, 512], F32, tag="pf1")
                    nc.tensor.matmul(
                        pf1, lhsT=h1,
                        rhs=x_t[:, g * 512:(g + 1) * 512],
                        start=True, stop=True)
                    nc.scalar.copy(zbig[:, g * 512:(g + 1) * 512], pf1)
                # ---- shuffle: chunk index onto partitions
                for c in range(CH):
                    nc.sync.dma_start(out=z2v[c], in_=zbv[:, c, :])
                # ---- F2 + fused had scale/shift evac
                y2t = y2p.tile([128, LOHI * t_blk], F32, tag="y2")
                for s in range(sl_per_blk):
                    pf2 = psf2.tile([128, 512], F32, tag="pf2")
                    nc.tensor.matmul(pf2, lhsT=s2,
                                     rhs=z2t[:, s * 512:(s + 1) * 512],
                                     start=True, stop=True)
                    for j in range(lohi_per_sl):
                        lohi = s * lohi_per_sl + j
                        nc.scalar.activation(
                            out=y2t[:, lohi * t_blk:(lohi + 1) * t_blk],
                            in_=pf2[:, j * t_blk:(j + 1) * t_blk],
                            func=AF.Identity,
                            scale=hs2p[:, lohi:lohi + 1],
                            bias=hb2[:, lohi:lohi + 1])
                # ---- gamma (all on-chip): abs-max over lohi, then over
                # partitions via PE transpose + free-dim reduce
                rmax = gp.tile([128, t_blk], F32, tag="rmax")
                nc.vector.tensor_reduce(
                    out=rmax,
                    in_=y2t.rearrange("p (h t) -> p t h", h=LOHI),
                    axis=mybir.AxisListType.X, op=AL.max,
                    apply_absolute_value=True)
                rmaxT = psf1.tile([128, 512], F32, tag="pf1")
                nc.tensor.transpose(rmaxT[0:t_blk, 0:128], rmax, id128)
                gam = gp.tile([t_blk, 1], F32, tag="gam")
                nc.vector.tensor_reduce(
                    out=gam, in_=rmaxT[0:t_blk, 0:128],
                    axis=mybir.AxisListType.X, op=AL.max)
                nc.vector.tensor_scalar_add(out=gam, in0=gam, scalar1=EPS)
                # per-token output scale gamma*wscale/7 (tokens on partitions)
                so = gp.tile([t_blk, 1], F32, tag="so")
                nc.vector.tensor_tensor(out=so, in0=gam, in1=ws7,
                                        op=AL.mult)
                # ---- sb = 7/gamma broadcast along partitions, t on free:
                # transpose gam -> [1, t], reciprocal*7, K=1 ones matmul
                gamTp = psf2.tile([1, 512], F32, tag="pf2")
                nc.tensor.transpose(gamTp[0:1, 0:t_blk], gam, id128)
                gamT = gp.tile([1, t_blk], F32, tag="gamT")
                nc.scalar.copy(gamT, gamTp[0:1, 0:t_blk])
                rec = gp.tile([1, t_blk], F32, tag="rec")
                nc.vector.reciprocal(out=rec, in_=gamT)
                nc.vector.tensor_scalar_mul(out=rec, in0=rec,
                                            scalar1=ACT_QB)
                sbp = psf1.tile([128, 512], F32, tag="pf1")
                nc.tensor.matmul(sbp[:, 0:t_blk], lhsT=ones1, rhs=rec,
                                 start=True, stop=True)
                sb = gp.tile([128, t_blk], F32, tag="sb")
                nc.scalar.copy(sb, sbp[:, 0:t_blk])
                # ---- quant: y2 *= sb (broadcast over lohi), round, cast fp8
                y2v = y2t.rearrange("p (h t) -> p h t", h=LOHI)
                sbv = sb.rearrange("p (o t) -> p o t", o=1)
                a1, a2 = bass.broadcast_tensor_aps(y2v, sbv)
                nc.vector.tensor_tensor(out=y2v, in0=a1, in1=a2,
                                        op=AL.mult)
                qb = qp.tile([128, CH * t_blk], FP8, tag="q")
                nc.vector.tensor_scalar(out=qb, in0=y2t, scalar1=MAGIC,
                                        scalar2=MAGIC, op0=AL.add,
                                        op1=AL.subtract)
                # ---- main matmul (fp8, chunk-pair DoubleRow)
                psums = [psm.tile([128, obw], F32, tag=f"pm{ob}",
                                  name=f"pm{ob}_{blk}")
                         for ob in range(n_ob)]
                if use_dr:
                    DR = mybir.MatmulPerfMode.DoubleRow
                    for kp in range(n_pair):
                        lhs3 = qb[:, (2 * kp) * t_blk:(2 * kp + 2) * t_blk
                                  ].rearrange("p (j t) -> p j t", j=2)
                        wg3 = wg_tiles[kp].rearrange("p (j n) -> p j n", j=2)
                        for ob in range(n_ob):
                            nc.tensor.matmul(
                                psums[ob], lhsT=lhs3,
                                rhs=wg3[:, :, ob * obw:(ob + 1) * obw],
                                start=(kp == 0), stop=(kp == n_pair - 1),
                                perf_mode=DR)
                else:
                    for kk in range(CH):
                        lhs = qb[:, kk * t_blk:(kk + 1) * t_blk]
                        wv = wg_tiles[kk // 2][
                            :, (kk % 2) * osh:(kk % 2) * osh + osh]
                        for ob in range(n_ob):
                            nc.tensor.matmul(
                                psums[ob], lhsT=lhs,
                                rhs=wv[:, ob * obw:(ob + 1) * obw],
                                start=(kk == 0), stop=(kk == CH - 1))
                ot = op_.tile([128, osh], F32, tag="ot")
                for ob in range(n_ob):
                    nc.scalar.activation(
                        out=ot[:, ob * obw:(ob + 1) * obw],
                        in_=psums[ob], func=AF.Copy, scale=so)
                nc.sync.dma_start(out=out_d[t0:t0 + t_blk, :], in_=ot)

                if debug_taps and blk == 0:
                    with tc.tile_pool(name="tapp", bufs=1) as tp:
                        nc.sync.dma_start(out=taps["zbig"][:, :], in_=zbig)
                        qbf = tp.tile([128, CH * t_blk], F32, tag="qf")
                        nc.vector.tensor_copy(qbf, qb)
                        nc.sync.dma_start(out=taps["qb"][:, :], in_=qbf)
                        nc.sync.dma_start(out=taps["y2"][:, :], in_=y2t)
                        nc.sync.dma_start(out=taps["gam"][:, :], in_=gam)
                        nc.sync.dma_start(out=taps["sb"][:, :], in_=sb)
                        wgf = tp.tile([128, 2 * osh], F32, tag="qf")
                        nc.vector.tensor_copy(wgf, wg_tiles[0])
                        nc.sync.dma_start(out=taps["wg0"][:, :], in_=wgf)

    return nc


def _split_multi_waits(nc):
    """walrus's CTRL encoder fits one sem-wait per instruction; Tile can emit
    several (e.g. the kernel-tail drain). Hoist extras onto standalone
    InstEventSemaphore carriers inserted just before the instruction."""
    import copy

    m = nc.m
    new_module = copy.replace(m, functions=[])
    ctr = 0
    for function in m.functions:
        new_function = copy.replace(function, blocks=[])
        new_function.set_allocations_from_list(function.allocations)
        for block in function.blocks:
            new_insts = []
            for inst in block.instructions:
                si = inst.sync_info
                ow = list(si.on_wait) if si is not None and si.on_wait else []
                if len(ow) > 1:
                    for w in ow[:-1]:
                        ctr += 1
                        new_insts.append(mybir.InstEventSemaphore(
                            name=f"I-wsplit-{ctr}",
                            engine=inst.engine,
                            ins=[], outs=[],
                            sync_info=mybir.SyncInfo(on_wait=[w],
                                                     on_update=[])))
                    inst = copy.replace(
                        inst,
                        sync_info=mybir.SyncInfo(on_wait=[ow[-1]],
                                                 on_update=si.on_update))
                new_insts.append(inst)
            new_block = copy.replace(block, instructions=new_insts)
            new_function.blocks.append(new_block)
        new_module.functions.append(new_function)
    nc.m = new_module
    return ctr


def host_prep(x, weight, had_scale, had_shift, n_cores=N_CORES, osh=None):
    """Shard + re-layout inputs for the SPMD program. Layout prep only."""
    T = int(np.prod(x.shape[:-1]))
    osh = osh or weight.shape[0] // n_cores
    H128p, S2, perm = host_consts()
    xt = np.ascontiguousarray(x.reshape(T, IN).T)  # [4096, T]
    id128 = np.eye(128, dtype=np.float32)
    hs2 = np.ascontiguousarray(
        had_scale[perm].reshape(CH, 128).T)  # [128(p2), 32(kk)]
    hb2 = np.ascontiguousarray(had_shift[perm].reshape(CH, 128).T)
    in_maps = []
    for core in range(n_cores):
        wsh = weight[core * osh:(core + 1) * osh, :]  # [osh, IN]
        wt = np.ascontiguousarray(wsh[:, perm].T)     # [IN(perm j), osh]
        in_maps.append({
            "xt": xt, "wt": wt, "hs2": hs2, "hb2": hb2,
            "h128p": H128p, "s2": S2, "id128": id128,
        })
    return in_maps


_PROGRAM_CACHE = {}


def _get_program(key, **kwargs):
    if key not in _PROGRAM_CACHE:
        nc = build_program(**kwargs)
        _split_multi_waits(nc)
        _PROGRAM_CACHE[key] = nc
    return _PROGRAM_CACHE[key]


def _timed_pjrt(nc, in_maps, n_cores, ks=(1, 4, 8), reps=3):
    """Chained steady-state timing.

    Queues k kernel executions back-to-back (donated per-call output
    buffers, identical to the one-shot path) and blocks once at the end.
    The slope between two chain lengths is the true per-iteration kernel
    time, independent of the client<->device tunnel round-trip latency
    (~80ms here) that dominates any single blocking call.
    """
    import time

    import jax
    import jax.numpy as jnp
    from jax.sharding import Mesh, NamedSharding, PartitionSpec
    from jax.experimental.shard_map import shard_map

    from concourse import bass2jax, mybir as mb

    bass2jax.install_neuronx_cc_hook()

    partition_name = (nc.partition_id_tensor.name
                      if nc.partition_id_tensor else None)
    in_names, out_names, out_avals = [], [], []
    for alloc in nc.m.functions[0].allocations:
        if not isinstance(alloc, mb.MemoryLocationSet):
            continue
        name = alloc.memorylocations[0].name
        if alloc.kind == "ExternalInput":
            if name != partition_name:
                in_names.append(name)
        elif alloc.kind == "ExternalOutput":
            out_names.append(name)
            shape = tuple(alloc.tensor_shape)
            dtype = mb.dt.np(alloc.dtype)
            out_avals.append(jax.core.ShapedArray(shape, dtype))
    n_params = len(in_names)
    all_in_names = list(in_names) + list(out_names)
    if partition_name is not None:
        all_in_names.append(partition_name)

    def _body(*args):
        operands = list(args)
        if partition_name is not None:
            operands.append(bass2jax.partition_id_tensor())
        outs = bass2jax._bass_exec_p.bind(
            *operands,
            out_avals=tuple(out_avals),
            in_names=tuple(all_in_names),
            out_names=tuple(out_names),
            lowering_input_output_aliases=(),
            sim_require_finite=True,
            sim_require_nnan=True,
            nc=nc,
        )
        return tuple(outs)

    devices = jax.devices()[:n_cores]
    mesh = Mesh(np.asarray(devices), ("core",))
    spec = NamedSharding(mesh, PartitionSpec("core"))
    n_outs = len(out_names)
    donate = tuple(range(n_params, n_params + n_outs))
    sharded = jax.jit(
        shard_map(_body, mesh=mesh,
                  in_specs=(PartitionSpec("core"),) * (n_params + n_outs),
                  out_specs=(PartitionSpec("core"),) * n_outs,
                  check_rep=False),
        donate_argnums=donate, keep_unused=True)

    concat_in = [
        np.concatenate([np.asarray(in_maps[c][nm]) for c in range(n_cores)],
                       axis=0)
        for nm in in_names
    ]
    dev_in = [jax.device_put(a, spec) for a in concat_in]
    zero_shapes = [(n_cores * z.shape[0], *z.shape[1:]) for z in out_avals]
    make_zeros = jax.jit(
        lambda: tuple(jnp.zeros(s, z.dtype)
                      for s, z in zip(zero_shapes, out_avals)),
        out_shardings=(spec,) * n_outs)

    dz = make_zeros()
    jax.block_until_ready(dz)
    out = sharded(*dev_in, *dz)  # warmup/compile
    jax.block_until_ready(out)

    totals = {}
    for k in ks:
        best = None
        for _ in range(reps):
            zsets = [make_zeros() for _ in range(k)]
            jax.block_until_ready(zsets)
            t0 = time.perf_counter()
            outs = [sharded(*dev_in, *zsets[i]) for i in range(k)]
            jax.block_until_ready(outs)
            dt = time.perf_counter() - t0
            best = dt if best is None else min(best, dt)
            out = outs[-1]
        totals[k] = best
        print(f"  chain k={k}: total {best*1e3:.2f} ms "
              f"({best/k*1e3:.2f} ms/call incl. tunnel RTT)", flush=True)
    ks_l = sorted(totals)
    k0, k1 = ks_l[0], ks_l[-1]
    per_iter = (totals[k1] - totals[k0]) / (k1 - k0) if k1 > k0 \
        else totals[k0]
    results = [
        {nm: np.asarray(out[i]).reshape(n_cores, *out_avals[i].shape)[c]
         for i, nm in enumerate(out_names)}
        for c in range(n_cores)
    ]
    return results, per_iter


def run(x, weight, had_scale, had_shift, trace=False, **trace_kwargs):
    x = np.asarray(x, dtype=np.float32)
    weight = np.asarray(weight, dtype=np.float32)
    had_scale = np.asarray(had_scale, dtype=np.float32)
    had_shift = np.asarray(had_shift, dtype=np.float32)
    batch_shape = x.shape[:-1]
    T = int(np.prod(batch_shape))
    nc = _get_program(("full", T), T=T)
    in_maps = host_prep(x, weight, had_scale, had_shift)
    if trace:
        results, per_iter = _timed_pjrt(nc, in_maps, N_CORES)
        times = [per_iter]
    else:
        res = run_bass_kernel_spmd(nc, in_maps, core_ids=list(range(N_CORES)))
        results, times = res.results, None
    shards = [results[c]["out"] for c in range(N_CORES)]
    out = np.concatenate(shards, axis=1).reshape(*batch_shape, OUT)
    return out, times


def kernel(**inputs):
    out, _ = run(inputs["x"], inputs["weight"], inputs["had_scale"],
                 inputs["had_shift"])
    return out


if __name__ == "__main__":
    # smoke: build only
    nc = build_program(T=256)
    n = sum(len(b.instructions) for f in nc.m.functions for b in f.blocks)
    print("build ok, insts:", n)


# revision 12
# speedup vs baseline: 15.5794x; 1.0686x over previous
"""Trainium2 Bass kernel for nn_EnhancedHBitLinear.

Computation (per reference.py):
  x [2, 4096, 4096] -> flatten tokens T=8192
  xh = FWHT_4096(x) / 64 * had_scale + had_shift
  gamma[t] = max|xh[t,:]| + 1e-5 ; q = round(xh * 7/gamma)  (int4 levels)
  wscale = mean|W| + 1e-5 ; tern = clip(round(W/wscale), -1, 1)
  out[t,o] = sum_i q[t,i]*tern[o,i] * (gamma[t]/7) * wscale

Sharding: Megatron column-parallel. weight split into 8 shards of 2048 output
features; x / had_* replicated. Each core computes the full activation
pipeline + its out-column shard; host concatenates shards.

Device dataflow (per core), features-on-partitions layout throughout:
  - host passes x transposed: xt [4096, 8192] (layout prep only)
  - weights: ternarized on device ({-1,0,1} via compares vs +-wscale/2),
        cast fp8e4, kept RESIDENT in SBUF as 16 chunk-pair tiles
        [128, 2*osh]; global wscale via AllReduce across 8 cores.
  - per 128-token block:
    - F1: 8 PE matmuls (H128 stationary, N=512 = 4 feature chunks) -> ACT
          evacuates into zbig [128, (c,t)]
    - shuffle: 32 SBUF->SBUF DMAs regroup partitions so chunk index is on
          partitions: z2[(c,lo4), (lohi,t)]
    - F2: 8 PE matmuls with (I4 (x) H32) -> full FWHT (feature-permuted);
          ACT evacuates PSUM fusing had_scale/64 (scale) + had_shift (bias)
    - gamma: DVE strided abs-max over lohi -> rmax [128,t]; PE transpose;
          DVE max over partitions -> gam [t,1] (+eps). All on-chip.
    - sb broadcast: PE transpose gam -> [1,t]; reciprocal*7; K=1 matmul
          with ones -> sb [128,t] (7/gamma broadcast over partitions)
    - quant: DVE mult y2*sb (stride-0 bcast over lohi) + round via
          +/-1.5*2^23 magic (round-half-even, matches jnp.round), cast fp8e4.
    - main matmul: fp8 DoubleRow (chunk pairs, contraction 256/pass) into
          4 PSUM banks; fp32 accumulation is EXACT (integer sums <= 7*4096);
          ACT evac applies gamma*wscale/7 per token (per-partition scale).

kernel(**inputs) is self-contained: hardcodes shapes, shards internally,
runs on 8 NeuronCores via run_bass_kernel_spmd, gathers the full output.
"""

import math
import os
import sys

import numpy as np

sys.path.insert(0, "/opt/trn_rl_repo")

import concourse.bass as bass
import concourse.mybir as mybir
import concourse.tile as tile
from concourse import bass_isa, library_config
from concourse.bass_utils import run_bass_kernel_spmd

F32 = mybir.dt.float32
FP8 = mybir.dt.float8e4

IN = 4096
OUT = 16384
N_CORES = 8
OSH = OUT // N_CORES  # 2048 out features per core
T_FULL = 8192

CH = 32   # feature chunks of 128 (IN/128)
LOHI = 32
LO4 = 4
EPS = 1e-5
ACT_QB = 7.0
MAGIC = 12582912.0  # 1.5 * 2**23 : v+M-M == round-half-even(v) for |v| < 2**22


def _hadamard(n):
    h = np.array([[1.0]], dtype=np.float32)
    while h.shape[0] < n:
        h = np.block([[h, h], [h, -h]])
    return h


def host_consts():
    """H128p: F1 stationary with output partitions permuted so partition
    m = lo4*32+lohi holds FWHT-low-bits index lo = lohi*4+lo4 (makes the
    shuffle DMA source slices partition-contiguous).
    S2: F2 stationary. S2[k=lo4p*32+cp, m=lo4o*32+co] = (lo4p==lo4o)*H32[co,cp].
    perm: flat feature permutation of the pipeline output:
    j = kk*128 + p2 (k-chunk kk=lohi, partition p2=lo4*32+c) -> original i."""
    H128 = _hadamard(128)
    m = np.arange(128)
    lo_of_m = (m % 32) * 4 + m // 32
    H128p = H128[:, lo_of_m].astype(np.float32).copy()

    H32 = _hadamard(32)
    # z2 partition p2 = c*4 + lo4 (contiguous partitions per chunk c)
    # S2[k = cp*4+lo4p, m = c*4+lo4] = (lo4p==lo4) * H32[c, cp]
    S2 = np.zeros((128, 128), dtype=np.float32)
    for cp in range(32):
        for c in range(32):
            for lo4 in range(LO4):
                S2[cp * 4 + lo4, c * 4 + lo4] = H32[c, cp]
    perm = np.zeros(IN, dtype=np.int64)
    for kk in range(CH):
        for p2 in range(128):
            c = p2 // 4
            lo4 = p2 % 4
            perm[kk * 128 + p2] = c * 128 + kk * 4 + lo4
    return H128p, S2, perm


def build_program(n_cores=N_CORES, T=T_FULL, osh=OSH, t_blk=128,
                  debug=False, use_collective=True, use_dr=True,
                  debug_taps=False):
    """Build the single SPMD Bass program (identical on all cores)."""
    assert T % t_blk == 0
    nblk = T // t_blk
    obw = 512
    n_ob = osh // obw
    sl_per_blk = (LOHI * t_blk) // 512  # 512-wide slices per block (F1/F2)
    lohi_per_sl = LOHI // sl_per_blk
    n_pair = CH // 2

    nc = bass.Bass("TRN2", target_bir_lowering=False, debug=debug,
                   num_devices=n_cores)

    taps = {}
    lite = os.environ.get("K_TAPS_LITE")
    if lite:
        taps["y2"] = nc.dram_tensor("tap_y2", [128, LOHI * t_blk], F32,
                                    kind="ExternalOutput")
        taps["gam"] = nc.dram_tensor("tap_gam", [t_blk, 1], F32,
                                     kind="ExternalOutput")
        taps["qb8"] = nc.dram_tensor("tap_qb8", [128, CH * t_blk], FP8,
                                     kind="ExternalOutput")
        taps["z2"] = nc.dram_tensor("tap_z2", [128, LOHI * t_blk], F32,
                                    kind="ExternalOutput")
    if debug_taps:
        taps["zbig"] = nc.dram_tensor("tap_zbig", [128, CH * t_blk], F32,
                                      kind="ExternalOutput")
        taps["y2"] = nc.dram_tensor("tap_y2", [128, LOHI * t_blk], F32,
                                    kind="ExternalOutput")
        taps["gam"] = nc.dram_tensor("tap_gam", [t_blk, 1], F32,
                                     kind="ExternalOutput")
        taps["sb"] = nc.dram_tensor("tap_sb", [128, t_blk], F32,
                                    kind="ExternalOutput")
        taps["qb"] = nc.dram_tensor("tap_qb", [128, CH * t_blk], F32,
                                    kind="ExternalOutput")
        taps["wg0"] = nc.dram_tensor("tap_wg0", [128, 2 * osh], F32,
                                     kind="ExternalOutput")

    xt_d = nc.dram_tensor("xt", [IN, T], F32, kind="ExternalInput")
    wt_d = nc.dram_tensor("wt", [IN, osh], F32, kind="ExternalInput")
    hs_d = nc.dram_tensor("hs2", [128, LOHI], F32, kind="ExternalInput")
    hb_d = nc.dram_tensor("hb2", [128, LOHI], F32, kind="ExternalInput")
    h1_d = nc.dram_tensor("h128p", [128, 128], F32, kind="ExternalInput")
    s2_d = nc.dram_tensor("s2", [128, 128], F32, kind="ExternalInput")
    id_d = nc.dram_tensor("id128", [128, 128], F32, kind="ExternalInput")
    out_d = nc.dram_tensor("out", [T, osh], F32, kind="ExternalOutput")

    AL = mybir.AluOpType
    AF = mybir.ActivationFunctionType

    with tile.TileContext(nc) as tc:
        from contextlib import ExitStack
        with ExitStack() as ctx:
            singles = ctx.enter_context(tc.tile_pool(name="singles", bufs=1))
            dramp = ctx.enter_context(
                tc.tile_pool(name="dramp", bufs=1, space=bass.MemorySpace.DRAM))

            cc_in = dramp.tile([1, 1], F32)
            cc_out = dramp.tile([1, 1], F32)
            # DRAM bounce buffers for the FWHT corner-turn (2-block ring)
            zdp = ctx.enter_context(
                tc.tile_pool(name="zdp", bufs=2, space=bass.MemorySpace.DRAM))

            # constants
            h1 = singles.tile([128, 128], F32)
            nc.sync.dma_start(out=h1, in_=h1_d[:, :])
            s2 = singles.tile([128, 128], F32)
            nc.sync.dma_start(out=s2, in_=s2_d[:, :])
            hs2 = singles.tile([128, LOHI], F32)
            nc.sync.dma_start(out=hs2, in_=hs_d[:, :])
            hb2 = singles.tile([128, LOHI], F32)
            nc.sync.dma_start(out=hb2, in_=hb_d[:, :])
            id128 = singles.tile([128, 128], F32)
            nc.sync.dma_start(out=id128, in_=id_d[:, :])
            ones1 = singles.tile([1, 128], F32)
            nc.vector.memset(ones1, 1.0)
            ones128 = singles.tile([128, 1], F32)
            nc.vector.memset(ones128, 1.0)
            hs2p = singles.tile([128, LOHI], F32)
            nc.vector.tensor_scalar_mul(out=hs2p, in0=hs2, scalar1=1.0 / 64.0)

            # resident ternarized weights: 16 chunk-pair tiles [128, 2*osh]
            wgp = ctx.enter_context(tc.tile_pool(name="wgp", bufs=1))
            wg_tiles = [wgp.tile([128, 2 * osh], FP8, tag=f"wg{i}",
                                 name=f"wg{i}")
                        for i in range(n_pair)]

            # ---------------- weight prep ----------------
            scl128 = singles.tile([128, 1], F32)   # wscale bcast
            thr128 = singles.tile([128, 1], F32)   # +wscale/2
            nthr128 = singles.tile([128, 1], F32)  # -wscale/2
            ws7 = singles.tile([128, 1], F32)      # wscale/7
            with tc.tile_pool(name="wprep", bufs=2) as wp, \
                 tc.tile_pool(name="wacc", bufs=1) as wa:
                accs = wa.tile([128, CH], F32)
                for kk in range(CH):
                    wf = wp.tile([128, osh], F32, tag="wf")
                    nc.scalar.dma_start(out=wf,
                                        in_=wt_d[kk * 128:(kk + 1) * 128, :])
                    nc.vector.tensor_reduce(
                        out=accs[:, kk:kk + 1], in_=wf,
                        axis=mybir.AxisListType.X, op=AL.add,
                        apply_absolute_value=True)
                tot128 = wa.tile([128, 1], F32)
                nc.vector.tensor_reduce(out=tot128, in_=accs,
                                        axis=mybir.AxisListType.X, op=AL.add)
                with tc.tile_pool(name="wps", bufs=1, space="PSUM") as wps:
                    tot1p = wps.tile([1, 1], F32)
                    nc.tensor.matmul(tot1p, lhsT=ones128, rhs=tot128,
                                     start=True, stop=True)
                    tot1 = wa.tile([1, 1], F32)
                    nc.scalar.copy(tot1, tot1p)
                nc.sync.dma_start(out=cc_in[:, :], in_=tot1)
                if use_collective:
                    nc.gpsimd.collective_compute(
                        "AllReduce", AL.add,
                        replica_groups=[list(range(n_cores))],
                        ins=[cc_in[:, :]], outs=[cc_out[:, :]])
                else:
                    nc.sync.dma_start(out=cc_out[:, :], in_=cc_in[:, :])
                totg = wa.tile([128, 1], F32)
                cc_bcast = bass.AP(tensor=cc_out.tensor, offset=cc_out.offset,
                                   ap=[[0, 128], [1, 1]])
                nc.gpsimd.dma_start(out=totg, in_=cc_bcast)
                # wscale = mean + EPS ; mean over the FULL weight
                nc.vector.tensor_scalar(
                    out=scl128, in0=totg, scalar1=1.0 / (IN * osh * n_cores),
                    scalar2=EPS, op0=AL.mult, op1=AL.add)
                nc.vector.tensor_scalar_mul(out=thr128, in0=scl128, scalar1=0.5)
                nc.vector.tensor_scalar_mul(out=nthr128, in0=scl128,
                                            scalar1=-0.5)
                nc.vector.tensor_scalar_mul(out=ws7, in0=scl128,
                                            scalar1=1.0 / 7.0)
                for kk in range(CH):
                    wf = wp.tile([128, osh], F32, tag="wf")
                    nc.scalar.dma_start(out=wf,
                                        in_=wt_d[kk * 128:(kk + 1) * 128, :])
                    g = wp.tile([128, osh], F32, tag="g")
                    nc.vector.tensor_scalar(out=g, in0=wf, scalar1=thr128,
                                            scalar2=None, op0=AL.is_gt)
                    l = wp.tile([128, osh], F32, tag="l")
                    nc.vector.tensor_scalar(out=l, in0=wf, scalar1=nthr128,
                                            scalar2=None, op0=AL.is_lt)
                    dst = wg_tiles[kk // 2][:, (kk % 2) * osh:(kk % 2 + 1) * osh]
                    nc.vector.tensor_tensor(out=dst, in0=g, in1=l,
                                            op=AL.subtract)

            # ---------------- main pipeline ----------------
            dbufs = 1 if debug_taps else int(os.environ.get("K_DBUFS", "2"))
            xp = ctx.enter_context(tc.tile_pool(name="xp", bufs=dbufs))
            zp = ctx.enter_context(tc.tile_pool(name="zp", bufs=1))
            z2p = ctx.enter_context(tc.tile_pool(name="z2p", bufs=dbufs))
            y2p = ctx.enter_context(tc.tile_pool(name="y2p", bufs=dbufs))
            qp = ctx.enter_context(tc.tile_pool(name="qp", bufs=2))
            op_ = ctx.enter_context(tc.tile_pool(name="op", bufs=2))
            gp = ctx.enter_context(tc.tile_pool(name="gp", bufs=2))
            psf1 = ctx.enter_context(
                tc.tile_pool(name="psf1", bufs=2, space="PSUM"))
            psf2 = ctx.enter_context(
                tc.tile_pool(name="psf2", bufs=2, space="PSUM"))
            psm = ctx.enter_context(
                tc.tile_pool(name="psm", bufs=1, space="PSUM"))

            for blk in range(nblk):
                t0 = blk * t_blk
                # ---- x load: one DMA for all 32 chunks of this block
                x_t = xp.tile([128, CH * t_blk], F32, tag="x")
                nc.sync.dma_start(
                    out=x_t.rearrange("p (c t) -> p c t", c=CH),
                    in_=xt_d[:, t0:t0 + t_blk].rearrange(
                        "(cc p) t -> p cc t", p=128))
                # ---- F1: 8 matmuls of N=512 (4 chunks each)
                zbig = zp.tile([128, CH * t_blk], F32, tag="zbig")
                zbv = zbig.rearrange("p (c t) -> p c t", c=CH)
                z2t = z2p.tile([128, LOHI * t_blk], F32, tag="z2")
                z2v = z2t.rearrange("(c f) (h t) -> c f h t",
                                    f=LO4, h=LOHI)
                for g in range(sl_per_blk):
                    pf1 = psf1.tile([128, 512], F32, tag="pf1")
                    nc.tensor.matmul(
                        pf1, lhsT=h1,
                        rhs=x_t[:, g * 512:(g + 1) * 512],
                        start=True, stop=True)
                    nc.scalar.copy(zbig[:, g * 512:(g + 1) * 512], pf1)
                # ---- corner-turn via DRAM bounce (2 full-partition DMAs):
                # scattered write puts chunk index on the row axis, plain
                # read brings it back on partitions.
                zd = zdp.tile([CH * LO4, LOHI * t_blk], F32, tag="zd")
                nc.scalar.dma_start(
                    out=zd.rearrange("(c l) (h t) -> l h c t",
                                     l=LO4, h=LOHI),
                    in_=zbv)
                nc.sync.dma_start(out=z2t, in_=zd[:, :])
                # ---- F2 + fused had scale/shift evac
                y2t = y2p.tile([128, LOHI * t_blk], F32, tag="y2")
                for s in range(sl_per_blk):
                    pf2 = psf2.tile([128, 512], F32, tag="pf2")
                    nc.tensor.matmul(pf2, lhsT=s2,
                                     rhs=z2t[:, s * 512:(s + 1) * 512],
                                     start=True, stop=True)
                    for j in range(lohi_per_sl):
                        lohi = s * lohi_per_sl + j
                        nc.scalar.activation(
                            out=y2t[:, lohi * t_blk:(lohi + 1) * t_blk],
                            in_=pf2[:, j * t_blk:(j + 1) * t_blk],
                            func=AF.Identity,
                            scale=hs2p[:, lohi:lohi + 1],
                            bias=hb2[:, lohi:lohi + 1])
                # ---- gamma (all on-chip): abs-max over lohi, then over
                # partitions via PE transpose + free-dim reduce
                rmax = gp.tile([128, t_blk], F32, tag="rmax")
                nc.vector.tensor_reduce(
                    out=rmax,
                    in_=y2t.rearrange("p (h t) -> p t h", h=LOHI),
                    axis=mybir.AxisListType.X, op=AL.max,
                    apply_absolute_value=True)
                rmaxT = psf1.tile([128, 512], F32, tag="pf1")
                nc.tensor.transpose(rmaxT[0:t_blk, 0:128], rmax, id128)
                gam = gp.tile([t_blk, 1], F32, tag="gam")
                nc.vector.tensor_reduce(
                    out=gam, in_=rmaxT[0:t_blk, 0:128],
                    axis=mybir.AxisListType.X, op=AL.max)
                nc.vector.tensor_scalar_add(out=gam, in0=gam, scalar1=EPS)
                # per-token output scale gamma*wscale/7 (tokens on partitions)
                so = gp.tile([t_blk, 1], F32, tag="so")
                nc.vector.tensor_tensor(out=so, in0=gam, in1=ws7,
                                        op=AL.mult)
                # ---- sb = 7/gamma broadcast along partitions, t on free:
                # transpose gam -> [1, t], reciprocal*7, K=1 ones matmul
                gamTp = psf2.tile([1, 512], F32, tag="pf2")
                nc.tensor.transpose(gamTp[0:1, 0:t_blk], gam, id128)
                gamT = gp.tile([1, t_blk], F32, tag="gamT")
                nc.scalar.copy(gamT, gamTp[0:1, 0:t_blk])
                rec = gp.tile([1, t_blk], F32, tag="rec")
                nc.vector.reciprocal(out=rec, in_=gamT)
                nc.vector.tensor_scalar_mul(out=rec, in0=rec,
                                            scalar1=ACT_QB)
                sbp = psf1.tile([128, 512], F32, tag="pf1")
                nc.tensor.matmul(sbp[:, 0:t_blk], lhsT=ones1, rhs=rec,
                                 start=True, stop=True)
                sb = gp.tile([128, t_blk], F32, tag="sb")
                nc.scalar.copy(sb, sbp[:, 0:t_blk])
                # ---- quant: y2 *= sb (broadcast over lohi), round, cast fp8
                y2v = y2t.rearrange("p (h t) -> p h t", h=LOHI)
                sbv = sb.rearrange("p (o t) -> p o t", o=1)
                a1, a2 = bass.broadcast_tensor_aps(y2v, sbv)
                nc.vector.tensor_tensor(out=y2v, in0=a1, in1=a2,
                                        op=AL.mult)
                qb = qp.tile([128, CH * t_blk], FP8, tag="q")
                nc.vector.tensor_scalar(out=qb, in0=y2t, scalar1=MAGIC,
                                        scalar2=MAGIC, op0=AL.add,
                                        op1=AL.subtract)
                # ---- main matmul (fp8, chunk-pair DoubleRow)
                psums = [psm.tile([128, obw], F32, tag=f"pm{ob}",
                                  name=f"pm{ob}_{blk}")
                         for ob in range(n_ob)]
                if use_dr:
                    DR = mybir.MatmulPerfMode.DoubleRow
                    for kp in range(n_pair):
                        lhs3 = qb[:, (2 * kp) * t_blk:(2 * kp + 2) * t_blk
                                  ].rearrange("p (j t) -> p j t", j=2)
                        wg3 = wg_tiles[kp].rearrange("p (j n) -> p j n", j=2)
                        for ob in range(n_ob):
                            nc.tensor.matmul(
                                psums[ob], lhsT=lhs3,
                                rhs=wg3[:, :, ob * obw:(ob + 1) * obw],
                                start=(kp == 0), stop=(kp == n_pair - 1),
                                perf_mode=DR)
                else:
                    for kk in range(CH):
                        lhs = qb[:, kk * t_blk:(kk + 1) * t_blk]
                        wv = wg_tiles[kk // 2][
                            :, (kk % 2) * osh:(kk % 2) * osh + osh]
                        for ob in range(n_ob):
                            nc.tensor.matmul(
                                psums[ob], lhsT=lhs,
                                rhs=wv[:, ob * obw:(ob + 1) * obw],
                                start=(kk == 0), stop=(kk == CH - 1))
                ot = op_.tile([128, osh], F32, tag="ot")
                for ob in range(n_ob):
                    nc.scalar.activation(
                        out=ot[:, ob * obw:(ob + 1) * obw],
                        in_=psums[ob], func=AF.Copy, scale=so)
                nc.sync.dma_start(out=out_d[t0:t0 + t_blk, :], in_=ot)

                if lite and blk == int(lite):
                    nc.sync.dma_start(out=taps["z2"][:, :], in_=z2t)
                    nc.sync.dma_start(out=taps["y2"][:, :], in_=y2t)
                    nc.sync.dma_start(out=taps["gam"][:, :], in_=gam)
                    nc.sync.dma_start(out=taps["qb8"][:, :], in_=qb)
                if debug_taps and blk == 0:
                    with tc.tile_pool(name="tapp", bufs=1) as tp:
                        nc.sync.dma_start(out=taps["zbig"][:, :], in_=zbig)
                        qbf = tp.tile([128, CH * t_blk], F32, tag="qf")
                        nc.vector.tensor_copy(qbf, qb)
                        nc.sync.dma_start(out=taps["qb"][:, :], in_=qbf)
                        nc.sync.dma_start(out=taps["y2"][:, :], in_=y2t)
                        nc.sync.dma_start(out=taps["gam"][:, :], in_=gam)
                        nc.sync.dma_start(out=taps["sb"][:, :], in_=sb)
                        wgf = tp.tile([128, 2 * osh], F32, tag="qf")
                        nc.vector.tensor_copy(wgf, wg_tiles[0])
                        nc.sync.dma_start(out=taps["wg0"][:, :], in_=wgf)

    return nc


def _split_multi_waits(nc):
    """walrus's CTRL encoder fits one sem-wait per instruction; Tile can emit
    several (e.g. the kernel-tail drain). Hoist extras onto standalone
    InstEventSemaphore carriers inserted just before the instruction."""
    import copy

    m = nc.m
    new_module = copy.replace(m, functions=[])
    ctr = 0
    for function in m.functions:
        new_function = copy.replace(function, blocks=[])
        new_function.set_allocations_from_list(function.allocations)
        for block in function.blocks:
            new_insts = []
            for inst in block.instructions:
                si = inst.sync_info
                ow = list(si.on_wait) if si is not None and si.on_wait else []
                if len(ow) > 1:
                    for w in ow[:-1]:
                        ctr += 1
                        new_insts.append(mybir.InstEventSemaphore(
                            name=f"I-wsplit-{ctr}",
                            engine=inst.engine,
                            ins=[], outs=[],
                            sync_info=mybir.SyncInfo(on_wait=[w],
                                                     on_update=[])))
                    inst = copy.replace(
                        inst,
                        sync_info=mybir.SyncInfo(on_wait=[ow[-1]],
                                                 on_update=si.on_update))
                new_insts.append(inst)
            new_block = copy.replace(block, instructions=new_insts)
            new_function.blocks.append(new_block)
        new_module.functions.append(new_function)
    nc.m = new_module
    return ctr


def host_prep(x, weight, had_scale, had_shift, n_cores=N_CORES, osh=None):
    """Shard + re-layout inputs for the SPMD program. Layout prep only."""
    T = int(np.prod(x.shape[:-1]))
    osh = osh or weight.shape[0] // n_cores
    H128p, S2, perm = host_consts()
    xt = np.ascontiguousarray(x.reshape(T, IN).T)  # [4096, T]
    id128 = np.eye(128, dtype=np.float32)
    hs2 = np.ascontiguousarray(
        had_scale[perm].reshape(CH, 128).T)  # [128(p2), 32(kk)]
    hb2 = np.ascontiguousarray(had_shift[perm].reshape(CH, 128).T)
    in_maps = []
    for core in range(n_cores):
        wsh = weight[core * osh:(core + 1) * osh, :]  # [osh, IN]
        wt = np.ascontiguousarray(wsh[:, perm].T)     # [IN(perm j), osh]
        in_maps.append({
            "xt": xt, "wt": wt, "hs2": hs2, "hb2": hb2,
            "h128p": H128p, "s2": S2, "id128": id128,
        })
    return in_maps


_PROGRAM_CACHE = {}


def _get_program(key, **kwargs):
    if key not in _PROGRAM_CACHE:
        nc = build_program(**kwargs)
        _split_multi_waits(nc)
        _PROGRAM_CACHE[key] = nc
    return _PROGRAM_CACHE[key]


def _timed_pjrt(nc, in_maps, n_cores, ks=(1, 4, 8), reps=3):
    """Chained steady-state timing.

    Queues k kernel executions back-to-back (donated per-call output
    buffers, identical to the one-shot path) and blocks once at the end.
    The slope between two chain lengths is the true per-iteration kernel
    time, independent of the client<->device tunnel round-trip latency
    (~80ms here) that dominates any single blocking call.
    """
    import time

    import jax
    import jax.numpy as jnp
    from jax.sharding import Mesh, NamedSharding, PartitionSpec
    from jax.experimental.shard_map import shard_map

    from concourse import bass2jax, mybir as mb

    bass2jax.install_neuronx_cc_hook()

    partition_name = (nc.partition_id_tensor.name
                      if nc.partition_id_tensor else None)
    in_names, out_names, out_avals = [], [], []
    for alloc in nc.m.functions[0].allocations:
        if not isinstance(alloc, mb.MemoryLocationSet):
            continue
        name = alloc.memorylocations[0].name
        if alloc.kind == "ExternalInput":
            if name != partition_name:
                in_names.append(name)
        elif alloc.kind == "ExternalOutput":
            out_names.append(name)
            shape = tuple(alloc.tensor_shape)
            dtype = mb.dt.np(alloc.dtype)
            out_avals.append(jax.core.ShapedArray(shape, dtype))
    n_params = len(in_names)
    all_in_names = list(in_names) + list(out_names)
    if partition_name is not None:
        all_in_names.append(partition_name)

    def _body(*args):
        operands = list(args)
        if partition_name is not None:
            operands.append(bass2jax.partition_id_tensor())
        outs = bass2jax._bass_exec_p.bind(
            *operands,
            out_avals=tuple(out_avals),
            in_names=tuple(all_in_names),
            out_names=tuple(out_names),
            lowering_input_output_aliases=(),
            sim_require_finite=True,
            sim_require_nnan=True,
            nc=nc,
        )
        return tuple(outs)

    devices = jax.devices()[:n_cores]
    mesh = Mesh(np.asarray(devices), ("core",))
    spec = NamedSharding(mesh, PartitionSpec("core"))
    n_outs = len(out_names)
    donate = tuple(range(n_params, n_params + n_outs))
    sharded = jax.jit(
        shard_map(_body, mesh=mesh,
                  in_specs=(PartitionSpec("core"),) * (n_params + n_outs),
                  out_specs=(PartitionSpec("core"),) * n_outs,
                  check_rep=False),
        donate_argnums=donate, keep_unused=True)

    concat_in = [
        np.concatenate([np.asarray(in_maps[c][nm]) for c in range(n_cores)],
                       axis=0)
        for nm in in_names
    ]
    dev_in = [jax.device_put(a, spec) for a in concat_in]
    zero_shapes = [(n_cores * z.shape[0], *z.shape[1:]) for z in out_avals]
    make_zeros = jax.jit(
        lambda: tuple(jnp.zeros(s, z.dtype)
                      for s, z in zip(zero_shapes, out_avals)),
        out_shardings=(spec,) * n_outs)

    dz = make_zeros()
    jax.block_until_ready(dz)
    out = sharded(*dev_in, *dz)  # warmup/compile
    jax.block_until_ready(out)

    totals = {}
    for k in ks:
        best = None
        for _ in range(reps):
            zsets = [make_zeros() for _ in range(k)]
            jax.block_until_ready(zsets)
            t0 = time.perf_counter()
            outs = [sharded(*dev_in, *zsets[i]) for i in range(k)]
            jax.block_until_ready(outs)
            dt = time.perf_counter() - t0
            best = dt if best is None else min(best, dt)
            out = outs[-1]
        totals[k] = best
        print(f"  chain k={k}: total {best*1e3:.2f} ms "
              f"({best/k*1e3:.2f} ms/call incl. tunnel RTT)", flush=True)
    ks_l = sorted(totals)
    k0, k1 = ks_l[0], ks_l[-1]
    per_iter = (totals[k1] - totals[k0]) / (k1 - k0) if k1 > k0 \
        else totals[k0]
    results = [
        {nm: np.asarray(out[i]).reshape(n_cores, *out_avals[i].shape)[c]
         for i, nm in enumerate(out_names)}
        for c in range(n_cores)
    ]
    return results, per_iter


def run(x, weight, had_scale, had_shift, trace=False, **trace_kwargs):
    x = np.asarray(x, dtype=np.float32)
    weight = np.asarray(weight, dtype=np.float32)
    had_scale = np.asarray(had_scale, dtype=np.float32)
    had_shift = np.asarray(had_shift, dtype=np.float32)
    batch_shape = x.shape[:-1]
    T = int(np.prod(batch_shape))
    nc = _get_program(("full", T), T=T)
    in_maps = host_prep(x, weight, had_scale, had_shift)
    if trace:
        results, per_iter = _timed_pjrt(nc, in_maps, N_CORES)
        times = [per_iter]
    else:
        res = run_bass_kernel_spmd(nc, in_maps, core_ids=list(range(N_CORES)))
        results, times = res.results, None
    shards = [results[c]["out"] for c in range(N_CORES)]
    out = np.concatenate(shards, axis=1).reshape(*batch_shape, OUT)
    return out, times


def kernel(**inputs):
    out, _ = run(inputs["x"], inputs["weight"], inputs["had_scale"],
                 inputs["had_shift"])
    return out


if __name__ == "__main__":
    # smoke: build only
    nc = build_program(T=256)
    n = sum(len(b.instructions) for f in nc.m.functions for b in f.blocks)
    print("build ok, insts:", n)
